# revision 30
# baseline (speedup 1.0000x reference)
"""Trainium2 Bass kernel: per-token int8 fake-quant x  @  int4-group-dequant W^T.

Math (matches torchao-style reference):
    x_dq = per_token_quant_dequant(x)            # [B*S, I]
    w_dq = (w_int - zeros) * scales per group    # [O, I]
    out  = x_dq @ w_dq.T                         # [B*S, O]

Device factorization:
    x_dq[t, i] = s[t] * qmz[t, i]   with qmz integer in [-255, 255] (exact in fp16)
    out[t, o]  = s[t] * sum_i qmz[t, i] * w_fp16[o, i]
qmz is computed with two fused tensor_scalar passes using the +1.5*2^23
round-to-nearest-even trick; w is dequantized on device to fp16 once and
stays resident in SBUF; per-token scale is applied to PSUM on readout.

Sharding: data-parallel over tokens, 8 cores x 1024 tokens each.
"""

from contextlib import ExitStack

import numpy as np

import concourse.bass as bass
import concourse.mybir as mybir
import concourse.tile as tile
from concourse import bass_utils
from concourse import masks

FP = mybir.dt.float32
BF = mybir.dt.bfloat16
F16 = mybir.dt.float16
I8 = mybir.dt.int8
ALU = mybir.AluOpType
ACTF = mybir.ActivationFunctionType

MAGIC = 12582912.0  # 1.5 * 2**23: add/sub forces RNE round-to-integer in fp32
EPS32 = float(np.finfo(np.float32).eps)
GROUP = 32

N_CORES = 8
B, S, D_IN, D_OUT = 4, 2048, 2048, 2048
TOK_FULL = B * S

MAX_WAITS_PER_INST = 1


def split_excess_waits(nc, max_waits=MAX_WAITS_PER_INST):
    """This walrus build rejects instructions with more than one sync-wait
    command. Move excess waits onto same-engine NOPs placed immediately
    before the over-subscribed instruction — semantically identical (the
    engine performs all waits before issuing)."""
    n_split = 0
    for f in nc.m.functions:
        for bb in f.blocks:
            insts = bb.instructions
            if not any(
                i.sync_info is not None and len(i.sync_info.on_wait or []) > max_waits
                for i in insts
            ):
                continue
            new = []
            for inst in insts:
                si = inst.sync_info
                waits = list(si.on_wait) if si is not None and si.on_wait else []
                if len(waits) > max_waits:
                    keep = waits[-max_waits:]
                    rest = waits[: len(waits) - max_waits]
                    for j in range(0, len(rest), max_waits):
                        nop = mybir.InstNoOp(
                            name=f"wsplit_{inst.name}_{j}",
                            engine=inst.engine,
                            ins=[],
                            outs=[],
                            sync_info=mybir.SyncInfo(
                                on_wait=rest[j : j + max_waits], on_update=[]
                            ),
                        )
                        new.append(nop)
                        n_split += 1
                    si.on_wait = keep
                new.append(inst)
            insts[:] = new
    return n_split


def build_nc(tok, d_in, d_out, wdt=F16, split_waits=True, tr_dma=True):
    """Single-pass fp16 kernel: resident dequantized weights, fused quant."""
    nt = tok // 128
    ni = d_in // 128
    noc = d_out // 512
    assert tok % 128 == 0 and d_in % 128 == 0 and d_out % 512 == 0

    nc = bass.Bass("TRN2", target_bir_lowering=False, debug=False)
    xs = nc.dram_tensor("xs", [tok, d_in], FP, kind="ExternalInput").ap()
    w8t = nc.dram_tensor("w8t", [d_in, d_out], I8, kind="ExternalInput").ap()
    # host-expanded per-element scales [d_in, d_out] fp32
    st = nc.dram_tensor("st", [d_in, d_out], FP, kind="ExternalInput").ap()
    out = nc.dram_tensor("out", [tok, d_out], FP, kind="ExternalOutput").ap()
    g_per_i = 128 // GROUP

    with tile.TileContext(nc) as tc, ExitStack() as ctx:
        const_pool = ctx.enter_context(tc.tile_pool(name="const", bufs=1))
        ident = const_pool.tile([128, 128], wdt, tag="ident", name="ident")
        masks.make_identity(nc, ident[:])
        magic_c = const_pool.tile([128, 1], FP, tag="magic", name="magic_c")
        nc.vector.memset(magic_c[:], MAGIC)

        stats = ctx.enter_context(tc.tile_pool(name="stats", bufs=1))
        xp = ctx.enter_context(tc.tile_pool(name="xp", bufs=3))
        qa_p = ctx.enter_context(tc.tile_pool(name="qa", bufs=2))
        qc_p = ctx.enter_context(tc.tile_pool(name="qc", bufs=2))
        qxt_p = ctx.enter_context(tc.tile_pool(name="qxt", bufs=1))
        stg_p = ctx.enter_context(tc.tile_pool(name="stg", bufs=1))
        w8_p = ctx.enter_context(tc.tile_pool(name="w8", bufs=4))
        sc_p = ctx.enter_context(tc.tile_pool(name="sc", bufs=3))
        wf_p = ctx.enter_context(tc.tile_pool(name="wf", bufs=1))
        out_p = ctx.enter_context(tc.tile_pool(name="outp", bufs=6))
        ps_mm = ctx.enter_context(tc.tile_pool(name="psmm", bufs=5, space="PSUM"))
        ps_tr = ctx.enter_context(tc.tile_pool(name="pstr", bufs=3, space="PSUM"))

        # ---- per-token quant chains (highest priority on DVE/GPSIMD)
        qxt = [
            qxt_p.tile([128, tok], wdt, tag=f"qxt{i}", name=f"qxt{i}")
            for i in range(ni)
        ]

        wf16 = [None] * ni
        GP_WF = set(range(ni // 2, ni))  # back half dequantized on GpSimd

        def _emit_wf_dma(i):
            w8 = w8_p.tile([128, d_out], I8, tag="w8", name=f"w8_{i}")
            nc.sync.dma_start(w8[:], w8t[i * 128 : (i + 1) * 128, :])
            sc = sc_p.tile([128, d_out], FP, tag="sc", name=f"sc_{i}")
            nc.scalar.dma_start(sc[:], st[i * 128 : (i + 1) * 128, :])
            return w8, sc

        wf_in = {}

        def _emit_wf_mul(i):
            w8, sc = wf_in[i]
            wf = wf_p.tile([128, d_out], wdt, tag=f"wf{i}", name=f"wf{i}")
            eng = nc.gpsimd if i in GP_WF else nc.vector
            eng.tensor_tensor(wf[:], w8[:], sc[:], ALU.mult)
            wf16[i] = wf

        s_tiles = []
        for t in range(nt):
            xt = xp.tile([128, d_in], FP, tag="xt", name=f"xt{t}")
            nc.sync.dma_start(xt[:], xs[t * 128 : (t + 1) * 128, :])
            # stream weight inputs alongside x: 2 low-i + 2 high-i per round
            for i in (2 * t, 2 * t + 1):
                if i < ni:
                    wf_in[i] = _emit_wf_dma(i)
            mn = stats.tile([128, 1], FP, tag=f"mn{t}", name=f"mn{t}")
            mx = stats.tile([128, 1], FP, tag=f"mx{t}", name=f"mx{t}")
            nc.vector.tensor_reduce(mn[:], xt[:], mybir.AxisListType.X, ALU.min)
            nc.vector.tensor_reduce(mx[:], xt[:], mybir.AxisListType.X, ALU.max)
            nc.vector.tensor_scalar(mn[:], mn[:], 0.0, None, ALU.min)
            nc.vector.tensor_scalar(mx[:], mx[:], 0.0, None, ALU.max)
            s_t = stats.tile([128, 1], FP, tag=f"s{t}", name=f"s{t}")
            nc.vector.tensor_tensor(s_t[:], mx[:], mn[:], ALU.subtract)
            nc.vector.tensor_scalar(
                s_t[:], s_t[:], float(np.float32(1.0) / np.float32(255.0)), EPS32,
                ALU.mult, ALU.max,
            )
            inv = stats.tile([128, 1], FP, tag=f"inv{t}", name=f"inv{t}")
            nc.vector.reciprocal(inv[:], s_t[:])
            u = stats.tile([128, 1], FP, tag=f"u{t}", name=f"u{t}")
            nc.vector.tensor_tensor(u[:], mn[:], inv[:], ALU.mult)
            nc.vector.tensor_scalar(u[:], u[:], MAGIC, None, ALU.add)
            nc.vector.tensor_scalar(u[:], u[:], MAGIC, None, ALU.subtract)
            c1 = stats.tile([128, 1], FP, tag=f"c1{t}", name=f"c1{t}")
            nc.vector.tensor_scalar(c1[:], u[:], MAGIC + 255.0, None, ALU.add)
            s_tiles.append(s_t)

            # qa = x*inv + M on GpSimd; qmz = min(qa, c1) - M -> fp16 on DVE
            qa = qa_p.tile([128, d_in], FP)
            nc.gpsimd.tensor_scalar(qa[:], xt[:], inv[:], MAGIC, ALU.mult, ALU.add)
            qc = qc_p.tile([128, d_in], wdt)
            nc.vector.tensor_scalar(qc[:], qa[:], c1[:], MAGIC, ALU.min, ALU.subtract)

            if tr_dma:
                for i in range(ni):
                    eng = nc.sync if i % 2 == 0 else nc.scalar
                    eng.dma_start_transpose(
                        qxt[i][:, t * 128 : (t + 1) * 128],
                        qc[:, i * 128 : (i + 1) * 128],
                    )
            else:
                for i in range(ni):
                    tr = ps_tr.tile([128, 128], wdt)
                    nc.tensor.transpose(
                        tr[:], qc[:, i * 128 : (i + 1) * 128], ident[:]
                    )
                    nc.scalar.activation(
                        qxt[i][:, t * 128 : (t + 1) * 128], tr[:], ACTF.Copy
                    )

            # weight-dequant multiplies as filler:
            # DVE handles low half (i = 2t, 2t+1 over first rounds),
            # GpSimd high half (8 + 2t, 9 + 2t)
            for i in (2 * t, 2 * t + 1):
                if i < ni // 2:
                    _emit_wf_mul(i)
            for i in (ni // 2 + 2 * t, ni // 2 + 2 * t + 1):
                if i < ni:
                    if i not in wf_in:
                        wf_in[i] = _emit_wf_dma(i)
                    _emit_wf_mul(i)

        # ---- matmul: for each token tile, accumulate over i with shared lhsT
        for t in range(nt):
            psums = [
                ps_mm.tile([128, 512], FP, tag="ps", name=f"ps_t{t}_{_oc}")
                for _oc in range(noc)
            ]
            for i in range(ni):
                lhs = qxt[i][:, t * 128 : (t + 1) * 128]
                for oc in range(noc):
                    nc.tensor.matmul(
                        psums[oc][:],
                        lhs,
                        wf16[i][:, oc * 512 : (oc + 1) * 512],
                        start=(i == 0),
                        stop=(i == ni - 1),
                    )
            for oc in range(noc):
                ot = out_p.tile([128, 512], FP)
                nc.scalar.mul(ot[:], psums[oc][:], s_tiles[t][:])
                nc.gpsimd.dma_start(
                    out[t * 128 : (t + 1) * 128, oc * 512 : (oc + 1) * 512],
                    ot[:],
                )
    if split_waits:
        split_excess_waits(nc)
    return nc


def _shard_inputs(x, w_int, w_scales, w_zeros, n_cores):
    tok = TOK_FULL // n_cores
    xf = np.ascontiguousarray(x.reshape(TOK_FULL, D_IN).astype(np.float32))
    w8t = np.ascontiguousarray(w_int.astype(np.int8).T)  # [I, O]
    # per-element scale, transposed+expanded: st[i, o] = w_scales[o, i//32]
    st = np.ascontiguousarray(
        np.repeat(w_scales.astype(np.float32).T, GROUP, axis=0)
    )  # [I, O]
    assert np.all(w_zeros == 0.0), "kernel assumes w_zeros == 0"
    in_maps = []
    for c in range(n_cores):
        in_maps.append(
            {"xs": xf[c * tok : (c + 1) * tok], "w8t": w8t, "st": st}
        )
    return in_maps


_NC_CACHE = {}


def _get_nc(wdt=F16):
    key = wdt
    if key not in _NC_CACHE:
        _NC_CACHE[key] = build_nc(TOK_FULL // N_CORES, D_IN, D_OUT, wdt=wdt)
    return _NC_CACHE[key]


def _ensure_ntff_hook():
    """This container lacks the antenv.axon_hooks shim that exposes the
    NTFF profile hook; reconstruct it from trn_boot's ctypes path."""
    import sys
    import types

    try:
        from antenv.axon_hooks import get_axon_ntff_profile_hook  # noqa: F401

        return
    except ImportError:
        pass
    hook = None
    try:
        import trn_agent_boot.trn_boot as tb

        hook = tb._ntff_profile_via_ctypes("/opt/axon/libaxon_pjrt.so")
    except Exception:
        hook = None
    mod = types.ModuleType("antenv.axon_hooks")
    mod.get_axon_ntff_profile_hook = lambda: hook
    mod.set_axon_ntff_profile_hook = lambda h: None
    import antenv

    antenv.axon_hooks = mod
    sys.modules["antenv.axon_hooks"] = mod


def kernel(x, w_int, w_scales, w_zeros, _trace=False, _wdt=F16):
    if _trace:
        _ensure_ntff_hook()
    in_maps = _shard_inputs(x, w_int, w_scales, w_zeros, N_CORES)
    nc = _get_nc(_wdt)
    res = bass_utils.run_bass_kernel_spmd(
        nc, in_maps, core_ids=list(range(N_CORES)), trace=_trace
    )
    tok = TOK_FULL // N_CORES
    full = np.concatenate([res.results[c]["out"] for c in range(N_CORES)], axis=0)
    out = full.reshape(B, S, D_OUT).astype(np.float32)
    if _trace:
        return out, res
    return out


# revision 31
# speedup vs baseline: 1.6245x; 1.6245x over previous
"""Trainium2 Bass kernel: per-token int8 fake-quant x  @  int4-group-dequant W^T.

Math (matches torchao-style reference):
    x_dq = per_token_quant_dequant(x)            # [B*S, I]
    w_dq = (w_int - zeros) * scales per group    # [O, I]
    out  = x_dq @ w_dq.T                         # [B*S, O]

Device factorization:
    x_dq[t, i] = s[t] * qmz[t, i]   with qmz integer in [-255, 255] (exact in fp16)
    out[t, o]  = s[t] * sum_i qmz[t, i] * w_fp16[o, i]
qmz is computed with two fused tensor_scalar passes using the +1.5*2^23
round-to-nearest-even trick; w is dequantized on device to fp16 once and
stays resident in SBUF; per-token scale is applied to PSUM on readout.

Sharding: data-parallel over tokens, 8 cores x 1024 tokens each.
"""

from contextlib import ExitStack

import numpy as np

import concourse.bass as bass
import concourse.mybir as mybir
import concourse.tile as tile
from concourse import bass_utils
from concourse import masks

FP = mybir.dt.float32
BF = mybir.dt.bfloat16
F16 = mybir.dt.float16
I8 = mybir.dt.int8
ALU = mybir.AluOpType
ACTF = mybir.ActivationFunctionType

MAGIC = 12582912.0  # 1.5 * 2**23: add/sub forces RNE round-to-integer in fp32
EPS32 = float(np.finfo(np.float32).eps)
GROUP = 32

N_CORES = 8
B, S, D_IN, D_OUT = 4, 2048, 2048, 2048
TOK_FULL = B * S

MAX_WAITS_PER_INST = 1


def split_excess_waits(nc, max_waits=MAX_WAITS_PER_INST):
    """This walrus build rejects instructions with more than one sync-wait
    command. Move excess waits onto same-engine NOPs placed immediately
    before the over-subscribed instruction — semantically identical (the
    engine performs all waits before issuing)."""
    n_split = 0
    for f in nc.m.functions:
        for bb in f.blocks:
            insts = bb.instructions
            if not any(
                i.sync_info is not None and len(i.sync_info.on_wait or []) > max_waits
                for i in insts
            ):
                continue
            new = []
            for inst in insts:
                si = inst.sync_info
                waits = list(si.on_wait) if si is not None and si.on_wait else []
                if len(waits) > max_waits:
                    keep = waits[-max_waits:]
                    rest = waits[: len(waits) - max_waits]
                    for j in range(0, len(rest), max_waits):
                        nop = mybir.InstNoOp(
                            name=f"wsplit_{inst.name}_{j}",
                            engine=inst.engine,
                            ins=[],
                            outs=[],
                            sync_info=mybir.SyncInfo(
                                on_wait=rest[j : j + max_waits], on_update=[]
                            ),
                        )
                        new.append(nop)
                        n_split += 1
                    si.on_wait = keep
                new.append(inst)
            insts[:] = new
    return n_split


def build_nc(tok, d_in, d_out, wdt=F16, split_waits=True, tr_dma=False):
    """Single-pass fp16 kernel: resident dequantized weights, fused quant."""
    nt = tok // 128
    ni = d_in // 128
    noc = d_out // 512
    assert tok % 128 == 0 and d_in % 128 == 0 and d_out % 512 == 0

    nc = bass.Bass("TRN2", target_bir_lowering=False, debug=False)
    xs = nc.dram_tensor("xs", [tok, d_in], FP, kind="ExternalInput").ap()
    w8t = nc.dram_tensor("w8t", [d_in, d_out], I8, kind="ExternalInput").ap()
    # host-expanded per-element scales [d_in, d_out] fp32
    st = nc.dram_tensor("st", [d_in, d_out], FP, kind="ExternalInput").ap()
    out = nc.dram_tensor("out", [tok, d_out], FP, kind="ExternalOutput").ap()
    g_per_i = 128 // GROUP

    with tile.TileContext(nc) as tc, ExitStack() as ctx:
        const_pool = ctx.enter_context(tc.tile_pool(name="const", bufs=1))
        ident = const_pool.tile([128, 128], wdt, tag="ident", name="ident")
        masks.make_identity(nc, ident[:])
        magic_c = const_pool.tile([128, 1], FP, tag="magic", name="magic_c")
        nc.vector.memset(magic_c[:], MAGIC)

        stats = ctx.enter_context(tc.tile_pool(name="stats", bufs=1))
        xp = ctx.enter_context(tc.tile_pool(name="xp", bufs=3))
        qa_p = ctx.enter_context(tc.tile_pool(name="qa", bufs=2))
        qc_p = ctx.enter_context(tc.tile_pool(name="qc", bufs=2))
        qxt_p = ctx.enter_context(tc.tile_pool(name="qxt", bufs=1))
        stg_p = ctx.enter_context(tc.tile_pool(name="stg", bufs=1))
        w8_p = ctx.enter_context(tc.tile_pool(name="w8", bufs=4))
        sc_p = ctx.enter_context(tc.tile_pool(name="sc", bufs=3))
        wf_p = ctx.enter_context(tc.tile_pool(name="wf", bufs=1))
        out_p = ctx.enter_context(tc.tile_pool(name="outp", bufs=6))
        ps_mm = ctx.enter_context(tc.tile_pool(name="psmm", bufs=5, space="PSUM"))
        ps_tr = ctx.enter_context(tc.tile_pool(name="pstr", bufs=3, space="PSUM"))

        # ---- per-token quant chains (highest priority on DVE/GPSIMD)
        qxt = [
            qxt_p.tile([128, tok], wdt, tag=f"qxt{i}", name=f"qxt{i}")
            for i in range(ni)
        ]

        wf16 = [None] * ni
        GP_WF = set(range(ni // 2, ni))  # back half dequantized on GpSimd

        def _emit_wf_dma(i):
            w8 = w8_p.tile([128, d_out], I8, tag="w8", name=f"w8_{i}")
            nc.sync.dma_start(w8[:], w8t[i * 128 : (i + 1) * 128, :])
            sc = sc_p.tile([128, d_out], FP, tag="sc", name=f"sc_{i}")
            nc.scalar.dma_start(sc[:], st[i * 128 : (i + 1) * 128, :])
            return w8, sc

        wf_in = {}

        def _emit_wf_mul(i):
            w8, sc = wf_in[i]
            wf = wf_p.tile([128, d_out], wdt, tag=f"wf{i}", name=f"wf{i}")
            eng = nc.gpsimd if i in GP_WF else nc.vector
            eng.tensor_tensor(wf[:], w8[:], sc[:], ALU.mult)
            wf16[i] = wf

        s_tiles = []
        for t in range(nt):
            xt = xp.tile([128, d_in], FP, tag="xt", name=f"xt{t}")
            nc.sync.dma_start(xt[:], xs[t * 128 : (t + 1) * 128, :])
            # stream weight inputs alongside x: 2 low-i + 2 high-i per round
            for i in (2 * t, 2 * t + 1):
                if i < ni:
                    wf_in[i] = _emit_wf_dma(i)
            mn = stats.tile([128, 1], FP, tag=f"mn{t}", name=f"mn{t}")
            mx = stats.tile([128, 1], FP, tag=f"mx{t}", name=f"mx{t}")
            nc.vector.tensor_reduce(mn[:], xt[:], mybir.AxisListType.X, ALU.min)
            nc.vector.tensor_reduce(mx[:], xt[:], mybir.AxisListType.X, ALU.max)
            nc.vector.tensor_scalar(mn[:], mn[:], 0.0, None, ALU.min)
            nc.vector.tensor_scalar(mx[:], mx[:], 0.0, None, ALU.max)
            s_t = stats.tile([128, 1], FP, tag=f"s{t}", name=f"s{t}")
            nc.vector.tensor_tensor(s_t[:], mx[:], mn[:], ALU.subtract)
            nc.vector.tensor_scalar(
                s_t[:], s_t[:], float(np.float32(1.0) / np.float32(255.0)), EPS32,
                ALU.mult, ALU.max,
            )
            inv = stats.tile([128, 1], FP, tag=f"inv{t}", name=f"inv{t}")
            nc.vector.reciprocal(inv[:], s_t[:])
            u = stats.tile([128, 1], FP, tag=f"u{t}", name=f"u{t}")
            nc.vector.tensor_tensor(u[:], mn[:], inv[:], ALU.mult)
            nc.vector.tensor_scalar(u[:], u[:], MAGIC, None, ALU.add)
            nc.vector.tensor_scalar(u[:], u[:], MAGIC, None, ALU.subtract)
            c1 = stats.tile([128, 1], FP, tag=f"c1{t}", name=f"c1{t}")
            nc.vector.tensor_scalar(c1[:], u[:], MAGIC + 255.0, None, ALU.add)
            s_tiles.append(s_t)

            # qa = x*inv + M on GpSimd; qmz = min(qa, c1) - M -> fp16 on DVE
            qa = qa_p.tile([128, d_in], FP)
            nc.gpsimd.tensor_scalar(qa[:], xt[:], inv[:], MAGIC, ALU.mult, ALU.add)
            qc = qc_p.tile([128, d_in], wdt)
            nc.vector.tensor_scalar(qc[:], qa[:], c1[:], MAGIC, ALU.min, ALU.subtract)

            if tr_dma:
                for i in range(ni):
                    eng = nc.sync if i % 2 == 0 else nc.scalar
                    eng.dma_start_transpose(
                        qxt[i][:, t * 128 : (t + 1) * 128],
                        qc[:, i * 128 : (i + 1) * 128],
                    )
            else:
                for i in range(ni):
                    tr = ps_tr.tile([128, 128], wdt)
                    nc.tensor.transpose(
                        tr[:], qc[:, i * 128 : (i + 1) * 128], ident[:]
                    )
                    nc.scalar.activation(
                        qxt[i][:, t * 128 : (t + 1) * 128], tr[:], ACTF.Copy
                    )

            # weight-dequant multiplies as filler:
            # DVE handles low half (i = 2t, 2t+1 over first rounds),
            # GpSimd high half (8 + 2t, 9 + 2t)
            for i in (2 * t, 2 * t + 1):
                if i < ni // 2:
                    _emit_wf_mul(i)
            for i in (ni // 2 + 2 * t, ni // 2 + 2 * t + 1):
                if i < ni:
                    if i not in wf_in:
                        wf_in[i] = _emit_wf_dma(i)
                    _emit_wf_mul(i)

        # ---- matmul: for each token tile, accumulate over i with shared lhsT
        for t in range(nt):
            psums = [
                ps_mm.tile([128, 512], FP, tag="ps", name=f"ps_t{t}_{_oc}")
                for _oc in range(noc)
            ]
            for i in range(ni):
                lhs = qxt[i][:, t * 128 : (t + 1) * 128]
                for oc in range(noc):
                    nc.tensor.matmul(
                        psums[oc][:],
                        lhs,
                        wf16[i][:, oc * 512 : (oc + 1) * 512],
                        start=(i == 0),
                        stop=(i == ni - 1),
                    )
            for oc in range(noc):
                ot = out_p.tile([128, 512], FP)
                nc.scalar.mul(ot[:], psums[oc][:], s_tiles[t][:])
                nc.gpsimd.dma_start(
                    out[t * 128 : (t + 1) * 128, oc * 512 : (oc + 1) * 512],
                    ot[:],
                )
    if split_waits:
        split_excess_waits(nc)
    return nc


def _shard_inputs(x, w_int, w_scales, w_zeros, n_cores):
    tok = TOK_FULL // n_cores
    xf = np.ascontiguousarray(x.reshape(TOK_FULL, D_IN).astype(np.float32))
    w8t = np.ascontiguousarray(w_int.astype(np.int8).T)  # [I, O]
    # per-element scale, transposed+expanded: st[i, o] = w_scales[o, i//32]
    st = np.ascontiguousarray(
        np.repeat(w_scales.astype(np.float32).T, GROUP, axis=0)
    )  # [I, O]
    assert np.all(w_zeros == 0.0), "kernel assumes w_zeros == 0"
    in_maps = []
    for c in range(n_cores):
        in_maps.append(
            {"xs": xf[c * tok : (c + 1) * tok], "w8t": w8t, "st": st}
        )
    return in_maps


_NC_CACHE = {}


def _get_nc(wdt=F16):
    key = wdt
    if key not in _NC_CACHE:
        _NC_CACHE[key] = build_nc(TOK_FULL // N_CORES, D_IN, D_OUT, wdt=wdt)
    return _NC_CACHE[key]


def _ensure_ntff_hook():
    """This container lacks the antenv.axon_hooks shim that exposes the
    NTFF profile hook; reconstruct it from trn_boot's ctypes path."""
    import sys
    import types

    try:
        from antenv.axon_hooks import get_axon_ntff_profile_hook  # noqa: F401

        return
    except ImportError:
        pass
    hook = None
    try:
        import trn_agent_boot.trn_boot as tb

        hook = tb._ntff_profile_via_ctypes("/opt/axon/libaxon_pjrt.so")
    except Exception:
        hook = None
    mod = types.ModuleType("antenv.axon_hooks")
    mod.get_axon_ntff_profile_hook = lambda: hook
    mod.set_axon_ntff_profile_hook = lambda h: None
    import antenv

    antenv.axon_hooks = mod
    sys.modules["antenv.axon_hooks"] = mod


def kernel(x, w_int, w_scales, w_zeros, _trace=False, _wdt=F16):
    if _trace:
        _ensure_ntff_hook()
    in_maps = _shard_inputs(x, w_int, w_scales, w_zeros, N_CORES)
    nc = _get_nc(_wdt)
    res = bass_utils.run_bass_kernel_spmd(
        nc, in_maps, core_ids=list(range(N_CORES)), trace=_trace
    )
    tok = TOK_FULL // N_CORES
    full = np.concatenate([res.results[c]["out"] for c in range(N_CORES)], axis=0)
    out = full.reshape(B, S, D_OUT).astype(np.float32)
    if _trace:
        return out, res
    return out
